# revision 22
# baseline (speedup 1.0000x reference)
"""MetapathAggrLayer Trainium2 kernel — v15.

Per node n: e_m = leakyrelu(x[m,n,:].a), w = softmax(e), out = sum_m w_m x[m,n,:].
Data-parallel over N across 8 NeuronCores; nodes-on-partitions layout.
~490us/core: DMA-bound (458us HBM busy at ~350GB/s/core for the mandatory
128MB read + 32MB write) + ~30us pipeline fill/drain/framework overhead.
(Baseline v9 was 857us: ACT t-loop products + DVE combine, compute-bound.)

Engine split per macro-tile (128 partitions x T nodes; 30 macros of T=32
plus one tail macro of T=17 -> 125,056 nodes/core, 448 pad nodes total):
  Sync : load xa/xb(v) fp32 (two half DMAs, metapath pairs)
  GpSimd ring: store osb  (separate HWDGE ring so loads never queue
               behind stores; GpSimd engine itself stays idle — its 2-port
               ops would arbitrate an exclusive SBUF port pair with DVE)
  ACT  : evict psum(v-2)->osb fp32, cast x(v)->xh fp16, exp(v)
  DVE  : scan_a/lrelu_a/scan_b/lrelu_b(v)    [fp32 prefix-scan scores]
         sred/recip/w2(v-1)                  [exp(v-1) ready -> no stall]
         merged product y(v-1) = xh*w2-bcast [fp16 all-packed -> 2x_1p]
  PE   : accumulating identity matmuls (v-1): psum[:,c] += I.T @ y_m[:,c]
The last macro is unstaggered (post-stage in its own iteration) to
collapse the pipeline drain.

Key facts this build relies on (measured + cost-model source): DVE 2x_1p
needs ALL operands 2-byte, innermost stride +-1, count>=2 (broadcast
stride-0 innermost disqualifies — hence the pair-duplicated w2); ACT
scale APs are per-partition scalars only; matmul accumulates in PSUM with
start/stop; DMA cannot touch PSUM; fp16 products keep rel err ~7e-4;
run-to-run HW variance is +-20us (8 cores share chip HBM).
"""

import sys

sys.path.insert(0, "/opt/trn_rl_repo")

import numpy as np

import concourse.bacc as bacc
import concourse.mybir as mybir
from concourse import bass_utils, dve_ops
from concourse.dve_spec import Spec, Src0, Src1, C0, scan, maxx, AluOp, lower, _has_src1
from concourse.dve_uop import DveOpSpec
from concourse.tile import TileContext

ALPHA = 0.2
NMETA = 4
F = 64
N_FULL = 1_000_000
N_CORES = 8
T = 32                     # nodes per partition per full macro-tile
T_TAIL = 17                # nodes per partition in the tail macro
FULL_MACROS = 30
T_LIST = [T] * FULL_MACROS + [T_TAIL]
NC_NODES = 128 * sum(T_LIST)          # 125_056
N_PAD = N_CORES * NC_NODES            # 1_000_448
NSEG = NMETA * T           # score segments per partition per full macro
NM = T * F                 # per-metapath free elems (full macro)
NALL = NMETA * NM          # merged free elems (full macro)
NH = NALL // 2             # half (one metapath pair)
MM_CHUNK = 512             # matmul moving free-dim limit

_CACHE = {}


def _register_op(name, spec, subdim=False):
    if name in dve_ops._SUB_OPCODE_FOR_NAME:
        return next(o for o in dve_ops.OPS if o.name == name)
    row = dve_ops._CUSTOM_DVE_ROW_BASE + len(dve_ops.OPS)
    assert row < 0x20
    shas = {}
    for ver in ("v3", "v4"):
        s = DveOpSpec(name=name, opcode=row, uops=lower(spec, ver=ver),
                      rd1_en=_has_src1(spec))
        shas[ver] = s.sha(ver)
    op = dve_ops.DveOp(name, spec, subdim, shas)
    dve_ops.OPS.append(op)
    dve_ops.CUSTOM_DVE_SPECS[name] = spec
    dve_ops._SUB_OPCODE_FOR_NAME[name] = row
    return op


def _get_ops():
    scan_mul = _register_op(
        "MPA_SCAN_MUL",
        Spec(
            body=scan(AluOp.ADD, Src0 * Src1),
            reference=lambda in0, in1, *cs: np.cumsum(
                (in0.astype(np.float32)
                 * in1.astype(np.float32).reshape(in0.shape)), axis=-1
            ),
        ),
    )
    ext_lrelu = _register_op(
        "MPA_EXT_LRELU",
        Spec(
            body=(lambda d: maxx(d, d * C0))(Src0 - Src1),
            reference=lambda in0, in1, s0=0.0, *cs: np.maximum(
                in0 - in1, (in0 - in1) * s0),
        ),
    )
    return scan_mul, ext_lrelu


def _build_kernel(t_list=None):
    scan_mul, ext_lrelu = _get_ops()

    nc = bacc.Bacc("TRN2", target_bir_lowering=False, debug=False)
    f32 = mybir.dt.float32
    f16 = mybir.dt.float16

    if t_list is None:
        t_list = T_LIST
    M = len(t_list)
    los = [128 * sum(t_list[:v]) for v in range(M)]
    nodes = 128 * sum(t_list)

    x_in = nc.dram_tensor("input", (NMETA, nodes, F), f32, kind="ExternalInput").ap()
    a_rep_in = nc.dram_tensor("a_rep", (128, F), f32, kind="ExternalInput").ap()
    ident_in = nc.dram_tensor("ident", (128, 128), f16, kind="ExternalInput").ap()
    out = nc.dram_tensor("out", (nodes, F), f32, kind="ExternalOutput").ap()

    mult = mybir.AluOpType.mult
    add = mybir.AluOpType.add

    with TileContext(nc) as tc:
        with tc.tile_pool(name="const", bufs=1) as cpool, \
             tc.tile_pool(name="xpa", bufs=3) as xpool_a, \
             tc.tile_pool(name="xpb", bufs=3) as xpool_b, \
             tc.tile_pool(name="xh", bufs=2) as hpool, \
             tc.tile_pool(name="prod", bufs=2) as prpool, \
             tc.tile_pool(name="psum", bufs=2, space="PSUM") as pspool, \
             tc.tile_pool(name="osb", bufs=2) as opool, \
             tc.tile_pool(name="small", bufs=2) as spool:
            a_rep = cpool.tile([128, F], f32)
            nc.gpsimd.dma_start(out=a_rep[:, :], in_=a_rep_in)
            ident = cpool.tile([128, 128], f16)
            nc.gpsimd.dma_start(out=ident[:, :], in_=ident_in)
            P = cpool.tile([128, NH + 1], f32)
            nc.gpsimd.memset(P[:, 0:1], 0.0)

            tiles = {}

            def dims(v):
                t = t_list[v]
                return t, t * F, NMETA * t  # t, nm, nseg

            def emit_load(v):
                t_v, nm_v, _ = dims(v)
                lo = los[v]
                hi = lo + 128 * t_v
                d = tiles[v] = {"lo": lo, "hi": hi, "t": t_v}
                d["xa"] = xpool_a.tile([128, NH], f32, tag="xa", name="xa")
                d["xb"] = xpool_b.tile([128, NH], f32, tag="xb", name="xb")
                # Macro 0 loads per-metapath quarters: hazard tracking is
                # region-based, so the first quarter-scan starts as soon as
                # metapath 0 lands (cuts pipeline fill ~5us).
                nq = 2 if v == 0 else 1
                for k, xt in ((0, d["xa"]), (1, d["xb"])):
                    for q in range(nq):
                        m0 = 2 * k + q * (2 // nq)
                        mw = 2 // nq
                        src = x_in[:, lo:hi, :][m0:m0 + mw].rearrange(
                            "m (p t) f -> p m t f", p=128)
                        dst = xt[:, q * (2 // nq) * nm_v:
                                 (q * (2 // nq) + mw) * nm_v].rearrange(
                            "p (m t f) -> p m t f", m=mw, f=F)
                        nc.sync.dma_start(out=dst, in_=src)

            def emit_store(j):
                ds = tiles[j]
                nm_s = ds["t"] * F
                dst = out[ds["lo"]:ds["hi"], :].rearrange(
                    "(p t) f -> p (t f)", p=128)
                nc.gpsimd.dma_start(out=dst, in_=ds["osb"][:, :nm_s])
                del tiles[j]

            def emit_evict(j):
                dc = tiles[j]
                nm_c = dc["t"] * F
                dc["osb"] = opool.tile([128, NM], f32, tag="osb", name="osb")
                nc.scalar.copy(dc["osb"][:, :nm_c], dc["acc"][:, :nm_c])

            def emit_cast(v):
                d = tiles[v]
                _, nm_v, _ = dims(v)
                d["xh"] = hpool.tile([128, NALL], f16, tag="xh", name="xh")
                nc.scalar.copy(d["xh"][:, 0:2 * nm_v], d["xa"][:, :2 * nm_v])
                nc.scalar.copy(d["xh"][:, 2 * nm_v:4 * nm_v],
                               d["xb"][:, :2 * nm_v])

            def emit_scans(v):
                d = tiles[v]
                t_v, nm_v, nseg_v = dims(v)
                d["e"] = spool.tile([128, NSEG], f32, tag="e", name="e")
                nq = 2 if v == 0 else 1  # quarter scans for macro 0
                seg_q = 2 * t_v // nq
                len_q = seg_q * F
                a_bc = a_rep[:, :].rearrange(
                    "p (o f) -> p o f", o=1).broadcast_to([128, seg_q, F])
                for k, xt in ((0, d["xa"]), (1, d["xb"])):
                    for q in range(nq):
                        nc.vector._custom_dve(
                            scan_mul, out=P[:, 1:len_q + 1],
                            in0=xt[:, q * len_q:(q + 1) * len_q], in1=a_bc,
                        )
                        p_hi = P[:, 1:len_q + 1].rearrange(
                            "p (s f) -> p s f", f=F)[:, :, F - 1:F]
                        p_lo = P[:, 0:len_q].rearrange(
                            "p (s f) -> p s f", f=F)[:, :, 0:1]
                        nc.vector._custom_dve(
                            ext_lrelu,
                            out=d["e"][:, k * 2 * t_v + q * seg_q:
                                       k * 2 * t_v + (q + 1) * seg_q],
                            in0=p_hi, in1=p_lo, s0=ALPHA,
                        )

            def emit_exp(v):
                d = tiles[v]
                _, _, nseg_v = dims(v)
                d["u"] = spool.tile([128, NSEG], f32, tag="u", name="u")
                nc.scalar.activation(d["u"][:, :nseg_v], d["e"][:, :nseg_v],
                                     mybir.ActivationFunctionType.Exp)

            def emit_post(j):
                # softmax smalls + merged fp16 product + PE combine for macro j
                db = tiles[j]
                t_b, nm_b, nseg_b = dims(j)
                db["s"] = spool.tile([128, T], f32, tag="s", name="s")
                u_tm = db["u"][:, :nseg_b].rearrange("p (m t) -> p t m", m=NMETA)
                nc.vector.tensor_reduce(out=db["s"][:, :t_b], in_=u_tm,
                                        axis=mybir.AxisListType.X, op=add)
                db["r"] = spool.tile([128, T], f32, tag="r", name="r")
                nc.vector.reciprocal(db["r"][:, :t_b], db["s"][:, :t_b])
                # w2[p, (m t j)] = u[p, (m t)] * r[p, t], j in {0,1}
                # pair-duplicated so the product in1 AP ends packed [1,2]
                db["w2"] = spool.tile([128, NSEG * 2], f16, tag="w2", name="w2")
                u_b = db["u"][:, :nseg_b].rearrange(
                    "p (m t o) -> p m t o", m=NMETA, o=1).broadcast_to(
                    [128, NMETA, t_b, 2])
                r_b = db["r"][:, :t_b].rearrange(
                    "p (o t oo) -> p o t oo", o=1, oo=1).broadcast_to(
                    [128, NMETA, t_b, 2])
                w2_v = db["w2"][:, :nseg_b * 2].rearrange(
                    "p (m t j) -> p m t j", m=NMETA, j=2)
                nc.vector.tensor_tensor(out=w2_v, in0=u_b, in1=r_b, op=mult)
                # merged product — fp16 all-packed for the 2x_1p DVE rate;
                # w2 is contiguous over (m t), so one op covers all metapaths
                db["y"] = prpool.tile([128, NALL], f16, tag="y", name="y")
                x3 = db["xh"][:, :4 * nm_b].rearrange(
                    "p (s f2 j) -> p s f2 j", f2=F // 2, j=2)
                w3 = db["w2"][:, :nseg_b * 2].rearrange(
                    "p (s o j) -> p s o j", o=1, j=2).broadcast_to(
                    [128, nseg_b, F // 2, 2])
                y3 = db["y"][:, :4 * nm_b].rearrange(
                    "p (s f2 j) -> p s f2 j", f2=F // 2, j=2)
                nc.vector.tensor_tensor(out=y3, in0=x3, in1=w3, op=mult)
                # PE combine: psum[:, c] = sum_m y_m[:, c]
                db["acc"] = pspool.tile([128, NM], f32, tag="acc", name="acc")
                cs = 0
                while cs < nm_b:
                    ce = min(cs + MM_CHUNK, nm_b)
                    for m in range(NMETA):
                        nc.tensor.matmul(
                            db["acc"][:, cs:ce],
                            ident[:, :],
                            db["y"][:, m * nm_b + cs:m * nm_b + ce],
                            start=(m == 0), stop=(m == NMETA - 1),
                        )
                    cs = ce

            # Schedules: steady state runs macro j's post-stage in iter j+1,
            # evict in j+2, store in j+3. The LAST macro is unstaggered (its
            # post-stage runs in its own iteration) to collapse the drain.
            last = M - 1
            for v in range(M + 2):
                post = ([v - 1] if 1 <= v <= last else []) + (
                    [last] if v == last else [])
                evicts = ([v - 2] if 2 <= v <= last + 1 else []) + (
                    [last] if v == last + 1 else [])
                stores = ([v - 3] if 3 <= v <= last + 2 else []) + (
                    [last] if v == last + 2 else [])

                if v < M:
                    emit_load(v)
                for j in stores:
                    emit_store(j)
                for j in evicts:
                    emit_evict(j)
                if v < M:
                    emit_cast(v)
                    emit_scans(v)
                    emit_exp(v)
                for j in post:
                    emit_post(j)

    nc.compile()
    return nc


def kernel(input, a, _trace=False):
    input = np.ascontiguousarray(np.asarray(input, dtype=np.float32))
    a = np.asarray(a, dtype=np.float32).reshape(F)

    if "nc" not in _CACHE:
        _CACHE["nc"] = _build_kernel()
    nc = _CACHE["nc"]

    pad = N_PAD - input.shape[1]
    xp = np.concatenate(
        [input, np.zeros((NMETA, pad, F), np.float32)], axis=1
    ) if pad else input

    a_rep = np.tile(a[None, :], (128, 1)).astype(np.float32)
    ident = np.eye(128, dtype=np.float16)

    in_maps = []
    for c in range(N_CORES):
        sl = xp[:, c * NC_NODES:(c + 1) * NC_NODES, :]
        in_maps.append({"input": np.ascontiguousarray(sl), "a_rep": a_rep,
                        "ident": ident})

    res = bass_utils.run_bass_kernel_spmd(
        nc, in_maps, core_ids=list(range(N_CORES)), trace=_trace
    )
    outs = [res.results[c]["out"] for c in range(N_CORES)]
    full = np.concatenate(outs, axis=0)[:N_FULL]
    if _trace:
        return full, res
    return full
